# revision 43
# baseline (speedup 1.0000x reference)
"""Bass/Tile Trainium2 kernel for a 2-layer dense multi-head GAT over a batch
of B=8 independent subgraphs (2048 nodes each, equal contiguous segments).

Sharding: one subgraph per NeuronCore (8 cores), parameters replicated.

Algorithm (per core / subgraph, per attention layer):
  scores are rank-1:  e_ij = leaky_relu(s1_i + s2_j),  s1 = h@a1, s2 = h@a2.
  exp(leaky_relu(t)) is separable through the sign mask M_ij = [s1_i+s2_j>=0]:
      p_ij = M_ij e^{s1_i} e^{s2_j} + (1-M_ij) e^{a s1_i} e^{a s2_j}
  so softmax(e) @ h needs NO N^2 exp work:
      num_i = g_i (M @ u)_i - ((M @ v) - vtot)_i           (e^{a s1} cancels in
      u_j = e^{s2_j} [h_j|1],  v_j = e^{a s2_j} [h_j|1],    the Z ratio; g =
      out_i = num_i[:64] / num_i[64]                        e^{(1-a) s1})
  The N^2 work is one DVE compare pass per (layer, j-chunk) producing a full
  [128, N] 0/1 bf16 mask row reused by all four i-quarters, plus bf16 mask
  matmuls (single stream, no residual: the 2e-2 tolerance leaves plenty of
  room).  vtot seeds the PSUM accumulator (bf16 hi+res rows, K=2 matmul), so
  A[:, DEXT:] = M@v - vtot directly and the epilogue is one fused op chain:
  ACT evacuates A, Pool computes nsum = g*Au - Av', DVE does the normalize+elu
  min/max in two-op tensor_scalars at bf16 4x rate.
"""

from contextlib import ExitStack

import numpy as np

import concourse.bass as bass
import concourse.tile as tile
from concourse import bacc, mybir
from concourse.masks import make_identity

FP = mybir.dt.float32
BF = mybir.dt.bfloat16
AF = mybir.ActivationFunctionType
OP = mybir.AluOpType

B = 8
N = 2048
D = 64
H = 4
ALPHA = 0.2
P = 128
NCH = N // P  # 16 chunks of 128 nodes
DEXT = D + 1  # h plus ones column

# full mask rows generated on Pool (GpSimd) instead of DVE, per layer
POOL_MASK_JCS = (14, 15)
DEBUG = False


def _attention(nc, pools, scratch, s12, s1b, hext, g, uv, ltag, out_cb):
    """Dense-GAT attention layer: out = softmax(lrelu(s1_i+s2_j)) @ h.

    s12:  [P, NCH, 2] SBUF f32 (s1|s2 in node-chunk column layout)
    s1b:  [P, N] SBUF bf16 (s1 replicated across partitions, free dim = node)
    hext: [P, NCH, DEXT] SBUF bf16 (h natural, col D == 1.0)
    g:    [P, NCH] SBUF f32 (e^{(1-a) s1})
    uv:   [P, NCH, 2*DEXT] SBUF bf16 ([e^{s2} hext | e^{a s2} hext])
    out_cb(nsum, rz, q): consumes quarter q ([P,4,DEXT] f32 + [P,4] recip).
    """
    const, prep, mask_pool, wide, small, psA, psaux = pools
    ones_col_bf = scratch["ones_col_bf"]
    ones_row_bf = scratch["ones_row_bf"]

    # --- vtot row: [1, 130] = [0...0 | -sum_j v_j] as bf16 hi+res, stacked
    # [2, 130] via a DMA hop so each accumulator is seeded by one K=2 matmul
    # (PSUM accumulation is order-insensitive; seeding happens last) ---
    vt_ps = psaux.tile([1, DEXT], FP, tag="aux", name=f"vt{ltag}")
    for c in range(NCH):
        nc.tensor.matmul(vt_ps, ones_col_bf, uv[:, c, DEXT:],
                         start=(c == 0), stop=(c == NCH - 1))
    vrow_bf = prep.tile([1, 2 * DEXT], BF, tag="vrow_bf")
    nc.vector.memset(vrow_bf[:, 0:DEXT], 0.0)
    nc.vector.tensor_scalar(vrow_bf[:, DEXT:], vt_ps, -1.0, None, OP.mult)
    vres = prep.tile([1, DEXT], BF, tag="vres")
    nc.vector.scalar_tensor_tensor(vres, vt_ps, -1.0, vrow_bf[:, DEXT:],
                                   OP.mult, OP.subtract)
    vrow2 = prep.tile([2, 2 * DEXT], BF, tag="vrow2")
    nc.sync.dma_start(out=vrow2[0:1, :], in_=vrow_bf)
    nc.sync.dma_start(out=vrow2[1:2, 0:DEXT], in_=vrow_bf[:, 0:DEXT])
    nc.sync.dma_start(out=vrow2[1:2, DEXT:], in_=vres)

    # --- full-row masks [128 j, N i], one per j-chunk, reused by all four
    # i-quarters.  A couple of rows go to the otherwise-idle Pool engine. ---
    mask_rows = []
    for jc in range(NCH):
        mt = mask_pool.tile([P, N], BF, tag="mrow", name=f"m{ltag}_{jc}")
        eng = nc.gpsimd if jc in POOL_MASK_JCS else nc.vector
        eng.tensor_scalar(mt, s1b, s12[:, jc, 1:2], 0.0, OP.add, OP.is_ge)
        mask_rows.append(mt)
    if scratch.get("dbg_m0") is not None and ltag == "h0":
        nc.gpsimd.dma_start(out=scratch["dbg_m0"][:, :], in_=mask_rows[0])
        nc.sync.dma_start(out=scratch["dbg_g"][:, :], in_=g)

    # --- masked attention matmuls + per-quarter epilogue ---
    for q in range(4):  # quarters of the i (destination-node) axis
        # one PSUM bank per il: interleaved accumulation chains must not
        # share a bank
        A = [psA.tile([P, 2 * DEXT], FP, tag="A", name=f"A{ltag}_{q}_{il}")
             for il in range(4)]
        for jc in range(NCH):
            mt = mask_rows[jc]
            for il in range(4):
                sl = mt[:, q * 512 + il * P: q * 512 + (il + 1) * P]
                nc.tensor.matmul(A[il], sl, uv[:, jc, :],
                                 start=(jc == 0), stop=False)
        for il in range(4):
            nc.tensor.matmul(A[il], ones_row_bf[0:2, :], vrow2,
                             start=False, stop=True)
        # epilogue: ACT evacuates the v-half (STT may read only one PSUM
        # operand), then DVE folds g: nsum = g*Au - Av', freeing the A banks
        nsum = wide.tile([P, 4, DEXT], FP, tag="nsum", name=f"ns{ltag}_{q}")
        for il in range(4):
            ic = q * 4 + il
            w = small.tile([P, DEXT], FP, tag="w")
            nc.scalar.copy(w, A[il][:, DEXT:])
            nc.vector.scalar_tensor_tensor(
                nsum[:, il, :], A[il][:, 0:DEXT],
                g[:, ic:ic + 1], w, OP.mult, OP.subtract)
        if DEBUG and ltag == "h0":
            dbg = scratch["dbg_ns"]
            nc.sync.dma_start(out=dbg[q], in_=nsum)
        rz = small.tile([P, 4], FP, tag="rz")
        nc.vector.reciprocal(rz, nsum[:, :, D])
        out_cb(nsum, rz, q)


def _elu_norm_q(nc, wide, pool_comb, nsum, rz, q, dst, ltag):
    """dst[:, q*4:(q+1)*4, ...] = elu(nsum[:, k, 0:D] * rz[:, k]) fused:
    r/m two-op tensor_scalars (bf16 out, DVE), exp on ACT, combine on Pool."""
    r = wide.tile([P, 4, D], BF, tag="elu_r", name=f"er{ltag}{q}")
    m = wide.tile([P, 4, D], BF, tag="elu_m", name=f"em{ltag}{q}")
    for k in range(4):
        # split normalize+clamp across DVE and the lightly-loaded Pool
        eng_r = nc.vector if k < 2 else nc.gpsimd
        eng_m = nc.gpsimd if k < 2 else nc.vector
        eng_r.tensor_scalar(r[:, k, :], nsum[:, k, 0:D], rz[:, k:k + 1],
                            0.0, OP.mult, OP.max)
        eng_m.tensor_scalar(m[:, k, :], nsum[:, k, 0:D], rz[:, k:k + 1],
                            0.0, OP.mult, OP.min)
    e = wide.tile([P, 4, D], BF, tag="elu_e", name=f"ee{ltag}{q}")
    nc.scalar.activation(e, m, AF.Exp)
    # dst = (e + (-1)) + r   (scalar_tensor_tensor is not a legal Pool opcode)
    nc.vector.scalar_tensor_tensor(dst, e, -1.0, r, OP.add, OP.add)


def build_kernel():
    nc = bacc.Bacc("TRN2", target_bir_lowering=False, debug=False,
                   num_devices=B)

    x = nc.dram_tensor("x", [N, D], FP, kind="ExternalInput")
    W_heads = nc.dram_tensor("W_heads", [H, D, D], FP, kind="ExternalInput")
    a_heads = nc.dram_tensor("a_heads", [H, 2 * D], FP, kind="ExternalInput")
    W_out = nc.dram_tensor("W_out", [H * D, D], FP, kind="ExternalInput")
    a_out = nc.dram_tensor("a_out", [2 * D], FP, kind="ExternalInput")
    out = nc.dram_tensor("out", [N, D], FP, kind="ExternalOutput")
    dbg_xc = nc.dram_tensor("dbg_xc", [P, NCH, 2, 2, D], FP,
                            kind="ExternalOutput") if DEBUG else None
    dbg_o2 = nc.dram_tensor("dbg_o2", [P, NCH, D], FP,
                            kind="ExternalOutput") if DEBUG else None
    dbg_s12 = nc.dram_tensor("dbg_s12", [P, NCH, 2], FP,
                             kind="ExternalOutput") if DEBUG else None
    dbg_uv = nc.dram_tensor("dbg_uv", [P, NCH, 2 * DEXT], FP,
                            kind="ExternalOutput") if DEBUG else None
    dbg_s1b = nc.dram_tensor("dbg_s1b", [P, N], FP,
                             kind="ExternalOutput") if DEBUG else None
    dbg_ns = nc.dram_tensor("dbg_ns", [4, P, 4, DEXT], FP,
                            kind="ExternalOutput") if DEBUG else None
    dbg_A = nc.dram_tensor("dbg_A", [4, P, 4, 2 * DEXT], FP,
                           kind="ExternalOutput") if DEBUG else None
    dbg_m0 = nc.dram_tensor("dbg_m0", [P, N], FP,
                            kind="ExternalOutput") if DEBUG else None
    dbg_g = nc.dram_tensor("dbg_g", [P, NCH], FP,
                           kind="ExternalOutput") if DEBUG else None

    with tile.TileContext(nc) as tc, ExitStack() as ctx:
        const = ctx.enter_context(tc.tile_pool(name="const", bufs=1))
        prep = ctx.enter_context(tc.tile_pool(name="prep", bufs=3))
        mask_pool = ctx.enter_context(tc.tile_pool(name="mask", bufs=22))
        wide = ctx.enter_context(tc.tile_pool(name="wide", bufs=3))
        small = ctx.enter_context(tc.tile_pool(name="small", bufs=6))
        psA = ctx.enter_context(tc.tile_pool(name="psA", bufs=4, space="PSUM"))
        psaux = ctx.enter_context(tc.tile_pool(name="psaux", bufs=4, space="PSUM"))
        pools = (const, prep, mask_pool, wide, small, psA, psaux)

        ident = const.tile([P, P], FP)
        make_identity(nc, ident)
        ones128 = const.tile([P, P], FP)
        nc.vector.memset(ones128, 1.0)
        ones_col_bf = const.tile([P, 1], BF)
        nc.vector.memset(ones_col_bf, 1.0)
        ones_row_bf = const.tile([2, P], BF)
        nc.vector.memset(ones_row_bf, 1.0)
        scratch = {"ones128": ones128, "ones_col_bf": ones_col_bf,
                   "ones_row_bf": ones_row_bf, "dbg_ns": dbg_ns,
                   "dbg_A": dbg_A, "dbg_m0": dbg_m0, "dbg_g": dbg_g}

        # ---- load inputs (x in 4 pieces so transposes start early) ----
        x_sb = const.tile([P, NCH, D], FP)
        x_r = x.rearrange("(c p) d -> p c d", p=P)
        for r4 in range(4):
            nc.sync.dma_start(out=x_sb[:, r4 * 4:(r4 + 1) * 4, :],
                              in_=x_r[:, r4 * 4:(r4 + 1) * 4, :])
        Wh = const.tile([64, H, D], FP)
        nc.sync.dma_start(out=Wh, in_=W_heads.rearrange("h k d -> k h d"))
        WhT = const.tile([64, H, D], FP)
        nc.sync.dma_start(out=WhT, in_=W_heads.rearrange("h k d -> d h k"))
        a_sb = const.tile([64, H, 2], FP)
        nc.sync.dma_start(out=a_sb, in_=a_heads.rearrange("h (t k) -> k h t", t=2))
        Wo = const.tile([P, 2, D], FP)
        nc.sync.dma_start(out=Wo, in_=W_out.rearrange("(c k) d -> k c d", k=P))
        WoT = const.tile([64, 2, P], FP)
        nc.sync.dma_start(out=WoT, in_=W_out.rearrange("(c k) d -> d c k", k=P))
        ao = const.tile([64, 2], FP)
        nc.sync.dma_start(out=ao, in_=a_out.rearrange("(t k) -> k t", t=2))

        # bf16 weight shadows for the payload-path matmuls
        Wh_bf = const.tile([64, H, D], BF)
        nc.gpsimd.tensor_copy(Wh_bf, Wh)
        Wo_bf = const.tile([P, 2, D], BF)
        nc.gpsimd.tensor_copy(Wo_bf, Wo)

        # ---- xT via PE transposes; bf16 shadow (Pool, in pieces) ----
        xT = const.tile([64, N], FP)
        for c in range(NCH):
            tp = psaux.tile([64, P], FP, tag="aux")
            nc.tensor.transpose(tp, x_sb[:, c, :], ident)
            # alternate evac engines so ACT is free for the head-0 prep chain
            if c % 2 == 0:
                nc.vector.tensor_copy(xT[:, c * P:(c + 1) * P], tp)
            else:
                nc.scalar.copy(xT[:, c * P:(c + 1) * P], tp)
        xT_bf = const.tile([64, N], BF)
        for r in range(4):
            nc.gpsimd.tensor_copy(xT_bf[:, r * 512:(r + 1) * 512],
                                  xT[:, r * 512:(r + 1) * 512])

        # all heads' wa = W_h @ [a1|a2] upfront (re-association: s = x @ wa);
        # only needs the parameter DMAs, so it fills the startup bubble
        wa_all = const.tile([64, H, 2], FP)
        for h in range(H):
            wap = psaux.tile([64, 2], FP, tag="aux", name=f"wap{h}")
            nc.tensor.matmul(wap, WhT[:, h, :], a_sb[:, h, :], start=True,
                             stop=True)
            nc.scalar.copy(wa_all[:, h, :], wap)

        def shared_prep(ltag, s12, W_bf_parts, wa1_src):
            """exps + s1b + hext + uv for one attention layer.
            W_bf_parts: (xTbf_part, W_part) contraction pairs for hext;
            wa1_src: (xTbf_part, wa1-row) contraction pairs for s1b."""
            es2 = prep.tile([P, NCH], FP, tag="es2", name=f"es2_{ltag}")
            nc.scalar.activation(es2, s12[:, :, 1], AF.Exp)
            es02 = prep.tile([P, NCH], FP, tag="es02", name=f"es02_{ltag}")
            nc.scalar.activation(es02, s12[:, :, 1], AF.Exp, scale=ALPHA)
            g = prep.tile([P, NCH], FP, tag="g", name=f"g_{ltag}")
            nc.scalar.activation(g, s12[:, :, 0], AF.Exp, scale=1.0 - ALPHA)

            # s1b (bf16, mask input only): s1 row replicated via ones x wa1
            s1b = prep.tile([P, N], BF, tag="s1b", name=f"s1b_{ltag}")
            for r in range(4):
                ps = psaux.tile([P, 512], FP, tag="aux")
                for ki, (xbf, w1b) in enumerate(wa1_src):
                    nc.tensor.matmul(ps, w1b, xbf[:, r * 512:(r + 1) * 512],
                                     start=(ki == 0),
                                     stop=(ki == len(wa1_src) - 1))
                nc.scalar.copy(s1b[:, r * 512:(r + 1) * 512], ps)

            # h natural (+ones col), bf16, evacuated in 4-chunk batches
            hext = prep.tile([P, NCH, DEXT], BF, tag="hext", name=f"he_{ltag}")
            nc.vector.memset(hext[:, :, D], 1.0)
            for cg in range(4):
                hp = psaux.tile([P, 4, D], FP, tag="aux", name=f"hp{ltag}{cg}")
                for k in range(4):
                    c = cg * 4 + k
                    for ki, (xbf, wbf) in enumerate(W_bf_parts):
                        nc.tensor.matmul(hp[:, k, :],
                                         xbf[:, c * P:(c + 1) * P], wbf,
                                         start=(ki == 0),
                                         stop=(ki == len(W_bf_parts) - 1))
                nc.scalar.copy(hext[:, cg * 4:(cg + 1) * 4, 0:D], hp)

            # uv = [e^{s2} hext | e^{a s2} hext] (all bf16); v half on Pool
            uv = prep.tile([P, NCH, 2 * DEXT], BF, tag="uv", name=f"uv_{ltag}")
            for c in range(NCH):
                nc.vector.tensor_scalar(uv[:, c, 0:DEXT], hext[:, c, :],
                                        es2[:, c:c + 1], None, OP.mult)
                nc.gpsimd.tensor_scalar(uv[:, c, DEXT:], hext[:, c, :],
                                        es02[:, c:c + 1], None, OP.mult)
            if DEBUG and ltag == "h0":
                nc.sync.dma_start(out=dbg_s12[:, :, :], in_=s12)
                nc.gpsimd.dma_start(out=dbg_s1b[:, :], in_=s1b)
                nc.gpsimd.dma_start(out=dbg_uv[:, :, :], in_=uv)
            return s1b, hext, g, uv

        # ---- layer 1: four heads -> xc01/xc23 (split so the layer-2
        # transposes of head-pair 0/1 need not wait for heads 2/3) ----
        xc01 = const.tile([P, NCH, 2, D], FP)
        xc23 = const.tile([P, NCH, 2, D], FP)

        for h in range(H):
            wa = wa_all[:, h, :]
            # s12 columns (batched copies, 4 chunks per PSUM tile)
            s12 = prep.tile([P, NCH, 2], FP, tag="s12", name=f"s12_{h}")
            for cg in range(4):
                sp = psaux.tile([P, 8], FP, tag="aux", name=f"sp{h}_{cg}")
                for k in range(4):
                    c = cg * 4 + k
                    nc.tensor.matmul(sp[:, 2 * k:2 * k + 2],
                                     xT[:, c * P:(c + 1) * P], wa,
                                     start=True, stop=True)
                nc.scalar.copy(s12[:, cg * 4:(cg + 1) * 4, :], sp)

            wa1b = prep.tile([64, P], BF, tag="wa1b", name=f"wa1b_{h}")
            nc.vector.tensor_scalar(wa1b, ones128[0:64, :], wa[:, 0:1], None,
                                    OP.mult)
            s1b, hext, g, uv = shared_prep(
                f"h{h}", s12, [(xT_bf, Wh_bf[:, h, :])], [(xT_bf, wa1b)])

            def l1_out(nsum, rz, q, h=h):
                xc = xc01 if h < 2 else xc23
                dst = xc[:, q * 4:(q + 1) * 4, h % 2, :]
                _elu_norm_q(nc, wide, True, nsum, rz, q, dst, f"h{h}")

            _attention(nc, pools, scratch, s12, s1b, hext, g, uv, f"h{h}",
                       l1_out)

        if DEBUG:
            nc.sync.dma_start(out=dbg_xc[:, :, 0, :, :], in_=xc01)
            nc.sync.dma_start(out=dbg_xc[:, :, 1, :, :], in_=xc23)

        # ---- transpose xc -> xcT_bf [P, 2, N] (feature-major, bf16 only:
        # the 2e-2 tolerance admits bf16 layer-2 scores, and dropping the
        # fp32 copy frees 16KB/partition for mask-row prefetch) ----
        xcT_bf = const.tile([P, 2, N], BF)
        for c in range(NCH):
            for kc, xc in ((0, xc01), (1, xc23)):
                tp = psaux.tile([P, P], FP, tag="aux")
                nc.tensor.transpose(tp, xc[:, c, :, :], ident)
                # alternate evac engines: ACT is busy with the last heads'
                # epilogue work in this region
                if (c + kc) % 2 == 0:
                    nc.vector.tensor_copy(xcT_bf[:, kc, c * P:(c + 1) * P], tp)
                else:
                    nc.scalar.copy(xcT_bf[:, kc, c * P:(c + 1) * P], tp)

        # ---- layer 2 projections ----
        wa2 = prep.tile([P, 2, 2], FP, tag="wa2")
        wa2_bf = prep.tile([P, 2, 2], BF, tag="wa2_bf")
        for kc in range(2):
            wap = psaux.tile([P, 2], FP, tag="aux", name=f"wap2_{kc}")
            nc.tensor.matmul(wap, WoT[:, kc, :], ao, start=True, stop=True)
            nc.scalar.copy(wa2[:, kc, :], wap)
            nc.vector.tensor_copy(wa2_bf[:, kc, :], wa2[:, kc, :])

        s12_2 = prep.tile([P, NCH, 2], FP, tag="s12", name="s12_l2")
        for cg in range(4):
            sp = psaux.tile([P, 8], FP, tag="aux", name=f"sp2_{cg}")
            for k in range(4):
                c = cg * 4 + k
                for kc in range(2):
                    nc.tensor.matmul(sp[:, 2 * k:2 * k + 2],
                                     xcT_bf[:, kc, c * P:(c + 1) * P],
                                     wa2_bf[:, kc, :],
                                     start=(kc == 0), stop=(kc == 1))
            nc.scalar.copy(s12_2[:, cg * 4:(cg + 1) * 4, :], sp)

        wa1b2 = prep.tile([P, 2, P], BF, tag="wa1b2")
        for kc in range(2):
            nc.vector.tensor_scalar(wa1b2[:, kc, :], ones128, wa2[:, kc, 0:1],
                                    None, OP.mult)
        s1b_2, h2ext, g_2, uv_2 = shared_prep(
            "l2", s12_2,
            [(xcT_bf[:, 0, :], Wo_bf[:, 0, :]), (xcT_bf[:, 1, :], Wo_bf[:, 1, :])],
            [(xcT_bf[:, 0, :], wa1b2[:, 0, :]), (xcT_bf[:, 1, :], wa1b2[:, 1, :])])

        # ---- layer 2 attention + elu + log_softmax -> out (chunked DMA) ----
        out_r = out.rearrange("(c p) d -> p c d", p=P)
        o2_all = x_sb  # x_sb is dead after the startup transposes; reuse
        esum_all = const.tile([P, NCH], FP)

        def l2_out(nsum, rz, q):
            # per quarter: elu + raw exp-sum (elu output is <= ~20, so exp is
            # fp32-safe without max subtraction); Ln + final subtract deferred
            # so the Exp/Ln ACT tables swap once, not per quarter
            o2 = o2_all[:, q * 4:(q + 1) * 4, :]
            _elu_norm_q(nc, wide, False, nsum, rz, q, o2, "l2")
            escr = wide.tile([P, 4, D], FP, tag="escr", name=f"escr{q}")
            for k in range(4):
                ic = q * 4 + k
                nc.scalar.activation(escr[:, k, :], o2[:, k, :], AF.Exp,
                                     accum_out=esum_all[:, ic:ic + 1])

        _attention(nc, pools, scratch, s12_2, s1b_2, h2ext, g_2, uv_2, "l2",
                   l2_out)

        if DEBUG:
            nc.sync.dma_start(out=dbg_o2[:, :, :], in_=o2_all)

        lse = wide.tile([P, NCH], FP, tag="lse")
        nc.scalar.activation(lse, esum_all, AF.Ln)
        out_w = const.tile([P, NCH, D], FP)
        for q in range(4):
            qs = slice(q * 4, (q + 1) * 4)
            for k in range(4):
                ic = q * 4 + k
                eng = nc.vector if k % 2 == 0 else nc.gpsimd
                eng.tensor_scalar(out_w[:, ic, :], o2_all[:, ic, :],
                                  lse[:, ic:ic + 1], None, OP.subtract)
            nc.sync.dma_start(out=out_r[:, qs, :], in_=out_w[:, qs, :])

    nc.compile()
    return nc


_NC_CACHE = {}


def _make_runner(nc):
    """Build a cached sharded executable (run_bass_kernel_spmd re-traces
    jax.jit on every call; this jits once and reuses)."""
    import jax
    from jax.sharding import Mesh, PartitionSpec
    try:
        from jax.experimental.shard_map import shard_map
    except ImportError:
        from jax.shard_map import shard_map
    import concourse.mybir as mb
    from concourse import bass2jax

    bass2jax.install_neuronx_cc_hook()

    part_name = nc.partition_id_tensor.name if nc.partition_id_tensor else None
    in_names, out_names, out_avals = [], [], []
    for alloc in nc.m.functions[0].allocations:
        if not isinstance(alloc, mb.MemoryLocationSet):
            continue
        name = alloc.memorylocations[0].name
        if alloc.kind == "ExternalInput":
            if name != part_name:
                in_names.append(name)
        elif alloc.kind == "ExternalOutput":
            out_names.append(name)
            out_avals.append(jax.core.ShapedArray(
                tuple(alloc.tensor_shape), mb.dt.np(alloc.dtype)))
    n_params = len(in_names)
    all_names = in_names + out_names
    if part_name is not None:
        all_names = all_names + [part_name]

    def _body(*args):
        operands = list(args)
        if part_name is not None:
            operands.append(bass2jax.partition_id_tensor())
        return tuple(bass2jax._bass_exec_p.bind(
            *operands, out_avals=tuple(out_avals), in_names=tuple(all_names),
            out_names=tuple(out_names), lowering_input_output_aliases=(),
            sim_require_finite=True, sim_require_nnan=True, nc=nc))

    devices = jax.devices()[:B]
    mesh = Mesh(np.asarray(devices), ("core",))
    n_outs = len(out_names)
    sharded = jax.jit(
        shard_map(_body, mesh=mesh,
                  in_specs=(PartitionSpec("core"),) * (n_params + n_outs),
                  out_specs=(PartitionSpec("core"),) * n_outs,
                  check_rep=False),
        donate_argnums=tuple(range(n_params, n_params + n_outs)),
        keep_unused=True)

    def run(in_maps):
        concat_in = [
            np.concatenate([np.asarray(in_maps[c][nm])[None] for c in range(B)],
                           axis=0).reshape(B * in_maps[0][nm].shape[0],
                                           *in_maps[0][nm].shape[1:])
            for nm in in_names
        ]
        concat_zeros = [
            np.zeros((B * av.shape[0], *av.shape[1:]), av.dtype)
            for av in out_avals
        ]
        out_arrs = sharded(*concat_in, *concat_zeros)
        return [
            {nm: np.asarray(out_arrs[i]).reshape(B, *out_avals[i].shape)[c]
             for i, nm in enumerate(out_names)}
            for c in range(B)
        ]

    return run


def kernel(**inputs):
    h_states = np.ascontiguousarray(np.asarray(inputs["h_states"], dtype=np.float32))
    W_heads = np.ascontiguousarray(np.asarray(inputs["W_heads"], dtype=np.float32))
    a_heads = np.ascontiguousarray(np.asarray(inputs["a_heads"], dtype=np.float32))
    W_out = np.ascontiguousarray(np.asarray(inputs["W_out"], dtype=np.float32))
    a_out = np.ascontiguousarray(np.asarray(inputs["a_out"], dtype=np.float32))

    if "nc" not in _NC_CACHE:
        _NC_CACHE["nc"] = build_kernel()
        _NC_CACHE["run"] = _make_runner(_NC_CACHE["nc"])

    xs = h_states.reshape(B, N, D)
    in_maps = [
        {"x": xs[c], "W_heads": W_heads, "a_heads": a_heads,
         "W_out": W_out, "a_out": a_out}
        for c in range(B)
    ]
    results = _NC_CACHE["run"](in_maps)
    return np.concatenate([results[c]["out"] for c in range(B)], axis=0)


if __name__ == "__main__":
    # smoke test (self-contained: random inputs, shape/dtype check only)
    rng = np.random.default_rng(0)
    inputs = {
        "h_states": rng.standard_normal((B * N, D)).astype(np.float32),
        "W_heads": rng.standard_normal((H, D, D)).astype(np.float32) * 0.18,
        "a_heads": rng.standard_normal((H, 2 * D)).astype(np.float32) * 0.18,
        "W_out": rng.standard_normal((H * D, D)).astype(np.float32) * 0.09,
        "a_out": rng.standard_normal((2 * D,)).astype(np.float32) * 0.18,
        "seq_start_end": (np.arange(B, dtype=np.int32)[:, None] * N
                          + np.array([0, N], dtype=np.int32)[None, :]),
    }
    got = kernel(**inputs)
    print("kernel output", got.shape, got.dtype)


# revision 45
# speedup vs baseline: 1.0720x; 1.0720x over previous
"""Bass/Tile Trainium2 kernel for a 2-layer dense multi-head GAT over a batch
of B=8 independent subgraphs (2048 nodes each, equal contiguous segments).

Sharding: one subgraph per NeuronCore (8 cores), parameters replicated.

Algorithm (per core / subgraph, per attention layer):
  scores are rank-1:  e_ij = leaky_relu(s1_i + s2_j),  s1 = h@a1, s2 = h@a2.
  exp(leaky_relu(t)) is separable through the sign mask M_ij = [s1_i+s2_j>=0]:
      p_ij = M_ij e^{s1_i} e^{s2_j} + (1-M_ij) e^{a s1_i} e^{a s2_j}
  so softmax(e) @ h needs NO N^2 exp work:
      num_i = g_i (M @ u)_i - ((M @ v) - vtot)_i           (e^{a s1} cancels in
      u_j = e^{s2_j} [h_j|1],  v_j = e^{a s2_j} [h_j|1],    the Z ratio; g =
      out_i = num_i[:64] / num_i[64]                        e^{(1-a) s1})
  The N^2 work is one DVE compare pass per (layer, j-chunk) producing a full
  [128, N] 0/1 bf16 mask row reused by all four i-quarters, plus bf16 mask
  matmuls (single stream, no residual: the 2e-2 tolerance leaves plenty of
  room).  vtot seeds the PSUM accumulator (bf16 hi+res rows, K=2 matmul), so
  A[:, DEXT:] = M@v - vtot directly and the epilogue is one fused op chain:
  ACT evacuates A, Pool computes nsum = g*Au - Av', DVE does the normalize+elu
  min/max in two-op tensor_scalars at bf16 4x rate.
"""

from contextlib import ExitStack

import numpy as np

import concourse.bass as bass
import concourse.tile as tile
from concourse import bacc, mybir
from concourse.masks import make_identity

FP = mybir.dt.float32
BF = mybir.dt.bfloat16
AF = mybir.ActivationFunctionType
OP = mybir.AluOpType

B = 8
N = 2048
D = 64
H = 4
ALPHA = 0.2
P = 128
NCH = N // P  # 16 chunks of 128 nodes
DEXT = D + 1  # h plus ones column

# full mask rows generated on Pool (GpSimd) instead of DVE, per layer
POOL_MASK_JCS = (14, 15)
DEBUG = False


def _attention(nc, pools, scratch, s12, s1b, hext, g, uv, ltag, out_cb):
    """Dense-GAT attention layer: out = softmax(lrelu(s1_i+s2_j)) @ h.

    s12:  [P, NCH, 2] SBUF f32 (s1|s2 in node-chunk column layout)
    s1b:  [P, N] SBUF bf16 (s1 replicated across partitions, free dim = node)
    hext: [P, NCH, DEXT] SBUF bf16 (h natural, col D == 1.0)
    g:    [P, NCH] SBUF f32 (e^{(1-a) s1})
    uv:   [P, NCH, 2*DEXT] SBUF bf16 ([e^{s2} hext | e^{a s2} hext])
    out_cb(nsum, rz, q): consumes quarter q ([P,4,DEXT] f32 + [P,4] recip).
    """
    const, prep, mask_pool, wide, small, psA, psaux = pools
    ones_col_bf = scratch["ones_col_bf"]
    ones_row_bf = scratch["ones_row_bf"]

    # --- vtot row: [1, 130] = [0...0 | -sum_j v_j] as bf16 hi+res, stacked
    # [2, 130] via a DMA hop so each accumulator is seeded by one K=2 matmul
    # (PSUM accumulation is order-insensitive; seeding happens last) ---
    vt_ps = psaux.tile([1, DEXT], FP, tag="aux", name=f"vt{ltag}")
    for c in range(NCH):
        nc.tensor.matmul(vt_ps, ones_col_bf, uv[:, c, DEXT:],
                         start=(c == 0), stop=(c == NCH - 1))
    vrow_bf = prep.tile([1, 2 * DEXT], BF, tag="vrow_bf")
    nc.vector.memset(vrow_bf[:, 0:DEXT], 0.0)
    nc.vector.tensor_scalar(vrow_bf[:, DEXT:], vt_ps, -1.0, None, OP.mult)
    vres = prep.tile([1, DEXT], BF, tag="vres")
    nc.vector.scalar_tensor_tensor(vres, vt_ps, -1.0, vrow_bf[:, DEXT:],
                                   OP.mult, OP.subtract)
    vrow2 = prep.tile([2, 2 * DEXT], BF, tag="vrow2")
    nc.sync.dma_start(out=vrow2[0:1, :], in_=vrow_bf)
    nc.sync.dma_start(out=vrow2[1:2, 0:DEXT], in_=vrow_bf[:, 0:DEXT])
    nc.sync.dma_start(out=vrow2[1:2, DEXT:], in_=vres)

    # --- full-row masks [128 j, N i], one per j-chunk, reused by all four
    # i-quarters.  A couple of rows go to the otherwise-idle Pool engine. ---
    mask_rows = []
    for jc in range(NCH):
        mt = mask_pool.tile([P, N], BF, tag="mrow", name=f"m{ltag}_{jc}")
        eng = nc.gpsimd if jc in POOL_MASK_JCS else nc.vector
        eng.tensor_scalar(mt, s1b, s12[:, jc, 1:2], 0.0, OP.add, OP.is_ge)
        mask_rows.append(mt)
    if scratch.get("dbg_m0") is not None and ltag == "h0":
        nc.gpsimd.dma_start(out=scratch["dbg_m0"][:, :], in_=mask_rows[0])
        nc.sync.dma_start(out=scratch["dbg_g"][:, :], in_=g)

    # --- masked attention matmuls + per-quarter epilogue ---
    for q in range(4):  # quarters of the i (destination-node) axis
        # one PSUM bank per il: interleaved accumulation chains must not
        # share a bank
        A = [psA.tile([P, 2 * DEXT], FP, tag="A", name=f"A{ltag}_{q}_{il}")
             for il in range(4)]
        for jc in range(NCH):
            mt = mask_rows[jc]
            for il in range(4):
                sl = mt[:, q * 512 + il * P: q * 512 + (il + 1) * P]
                nc.tensor.matmul(A[il], sl, uv[:, jc, :],
                                 start=(jc == 0), stop=False)
        for il in range(4):
            nc.tensor.matmul(A[il], ones_row_bf[0:2, :], vrow2,
                             start=False, stop=True)
        # epilogue: ACT evacuates the v-half (STT may read only one PSUM
        # operand), then DVE folds g: nsum = g*Au - Av', freeing the A banks
        nsum = wide.tile([P, 4, DEXT], FP, tag="nsum", name=f"ns{ltag}_{q}")
        for il in range(4):
            ic = q * 4 + il
            w = small.tile([P, DEXT], FP, tag="w")
            nc.scalar.copy(w, A[il][:, DEXT:])
            nc.vector.scalar_tensor_tensor(
                nsum[:, il, :], A[il][:, 0:DEXT],
                g[:, ic:ic + 1], w, OP.mult, OP.subtract)
        if DEBUG and ltag == "h0":
            dbg = scratch["dbg_ns"]
            nc.sync.dma_start(out=dbg[q], in_=nsum)
        rz = small.tile([P, 4], FP, tag="rz")
        nc.vector.reciprocal(rz, nsum[:, :, D])
        out_cb(nsum, rz, q)


def _elu_norm_q(nc, wide, pool_comb, nsum, rz, q, dst, ltag):
    """dst[:, q*4:(q+1)*4, ...] = elu(nsum[:, k, 0:D] * rz[:, k]) fused:
    r/m two-op tensor_scalars (bf16 out, DVE), exp on ACT, combine on Pool."""
    r = wide.tile([P, 4, D], BF, tag="elu_r", name=f"er{ltag}{q}")
    m = wide.tile([P, 4, D], BF, tag="elu_m", name=f"em{ltag}{q}")
    for k in range(4):
        nc.vector.tensor_scalar(r[:, k, :], nsum[:, k, 0:D], rz[:, k:k + 1],
                                0.0, OP.mult, OP.max)
        nc.vector.tensor_scalar(m[:, k, :], nsum[:, k, 0:D], rz[:, k:k + 1],
                                0.0, OP.mult, OP.min)
    e = wide.tile([P, 4, D], BF, tag="elu_e", name=f"ee{ltag}{q}")
    nc.scalar.activation(e, m, AF.Exp)
    # dst = (e + (-1)) + r   (scalar_tensor_tensor is not a legal Pool opcode)
    nc.vector.scalar_tensor_tensor(dst, e, -1.0, r, OP.add, OP.add)


def build_kernel():
    nc = bacc.Bacc("TRN2", target_bir_lowering=False, debug=False,
                   num_devices=B)

    x = nc.dram_tensor("x", [N, D], FP, kind="ExternalInput")
    W_heads = nc.dram_tensor("W_heads", [H, D, D], FP, kind="ExternalInput")
    a_heads = nc.dram_tensor("a_heads", [H, 2 * D], FP, kind="ExternalInput")
    W_out = nc.dram_tensor("W_out", [H * D, D], FP, kind="ExternalInput")
    a_out = nc.dram_tensor("a_out", [2 * D], FP, kind="ExternalInput")
    out = nc.dram_tensor("out", [N, D], FP, kind="ExternalOutput")
    dbg_xc = nc.dram_tensor("dbg_xc", [P, NCH, 2, 2, D], FP,
                            kind="ExternalOutput") if DEBUG else None
    dbg_o2 = nc.dram_tensor("dbg_o2", [P, NCH, D], FP,
                            kind="ExternalOutput") if DEBUG else None
    dbg_s12 = nc.dram_tensor("dbg_s12", [P, NCH, 2], FP,
                             kind="ExternalOutput") if DEBUG else None
    dbg_uv = nc.dram_tensor("dbg_uv", [P, NCH, 2 * DEXT], FP,
                            kind="ExternalOutput") if DEBUG else None
    dbg_s1b = nc.dram_tensor("dbg_s1b", [P, N], FP,
                             kind="ExternalOutput") if DEBUG else None
    dbg_ns = nc.dram_tensor("dbg_ns", [4, P, 4, DEXT], FP,
                            kind="ExternalOutput") if DEBUG else None
    dbg_A = nc.dram_tensor("dbg_A", [4, P, 4, 2 * DEXT], FP,
                           kind="ExternalOutput") if DEBUG else None
    dbg_m0 = nc.dram_tensor("dbg_m0", [P, N], FP,
                            kind="ExternalOutput") if DEBUG else None
    dbg_g = nc.dram_tensor("dbg_g", [P, NCH], FP,
                           kind="ExternalOutput") if DEBUG else None

    with tile.TileContext(nc) as tc, ExitStack() as ctx:
        const = ctx.enter_context(tc.tile_pool(name="const", bufs=1))
        prep = ctx.enter_context(tc.tile_pool(name="prep", bufs=3))
        mask_pool = ctx.enter_context(tc.tile_pool(name="mask", bufs=22))
        wide = ctx.enter_context(tc.tile_pool(name="wide", bufs=3))
        small = ctx.enter_context(tc.tile_pool(name="small", bufs=6))
        psA = ctx.enter_context(tc.tile_pool(name="psA", bufs=4, space="PSUM"))
        psaux = ctx.enter_context(tc.tile_pool(name="psaux", bufs=4, space="PSUM"))
        pools = (const, prep, mask_pool, wide, small, psA, psaux)

        ident = const.tile([P, P], FP)
        make_identity(nc, ident)
        ones128 = const.tile([P, P], FP)
        nc.vector.memset(ones128, 1.0)
        ones_col_bf = const.tile([P, 1], BF)
        nc.vector.memset(ones_col_bf, 1.0)
        ones_row_bf = const.tile([2, P], BF)
        nc.vector.memset(ones_row_bf, 1.0)
        scratch = {"ones128": ones128, "ones_col_bf": ones_col_bf,
                   "ones_row_bf": ones_row_bf, "dbg_ns": dbg_ns,
                   "dbg_A": dbg_A, "dbg_m0": dbg_m0, "dbg_g": dbg_g}

        # ---- load inputs (x in 4 pieces so transposes start early) ----
        x_sb = const.tile([P, NCH, D], FP)
        x_r = x.rearrange("(c p) d -> p c d", p=P)
        for r4 in range(4):
            nc.sync.dma_start(out=x_sb[:, r4 * 4:(r4 + 1) * 4, :],
                              in_=x_r[:, r4 * 4:(r4 + 1) * 4, :])
        Wh = const.tile([64, H, D], FP)
        nc.sync.dma_start(out=Wh, in_=W_heads.rearrange("h k d -> k h d"))
        WhT = const.tile([64, H, D], FP)
        nc.sync.dma_start(out=WhT, in_=W_heads.rearrange("h k d -> d h k"))
        a_sb = const.tile([64, H, 2], FP)
        nc.sync.dma_start(out=a_sb, in_=a_heads.rearrange("h (t k) -> k h t", t=2))
        Wo = const.tile([P, 2, D], FP)
        nc.sync.dma_start(out=Wo, in_=W_out.rearrange("(c k) d -> k c d", k=P))
        WoT = const.tile([64, 2, P], FP)
        nc.sync.dma_start(out=WoT, in_=W_out.rearrange("(c k) d -> d c k", k=P))
        ao = const.tile([64, 2], FP)
        nc.sync.dma_start(out=ao, in_=a_out.rearrange("(t k) -> k t", t=2))

        # bf16 weight shadows for the payload-path matmuls
        Wh_bf = const.tile([64, H, D], BF)
        nc.gpsimd.tensor_copy(Wh_bf, Wh)
        Wo_bf = const.tile([P, 2, D], BF)
        nc.gpsimd.tensor_copy(Wo_bf, Wo)

        # ---- xT via PE transposes; bf16 shadow (Pool, in pieces) ----
        xT = const.tile([64, N], FP)
        for c in range(NCH):
            tp = psaux.tile([64, P], FP, tag="aux")
            nc.tensor.transpose(tp, x_sb[:, c, :], ident)
            # alternate evac engines so ACT is free for the head-0 prep chain
            if c % 2 == 0:
                nc.vector.tensor_copy(xT[:, c * P:(c + 1) * P], tp)
            else:
                nc.scalar.copy(xT[:, c * P:(c + 1) * P], tp)
        xT_bf = const.tile([64, N], BF)
        for r in range(4):
            nc.gpsimd.tensor_copy(xT_bf[:, r * 512:(r + 1) * 512],
                                  xT[:, r * 512:(r + 1) * 512])

        # all heads' wa = W_h @ [a1|a2] upfront (re-association: s = x @ wa);
        # only needs the parameter DMAs, so it fills the startup bubble
        wa_all = const.tile([64, H, 2], FP)
        for h in range(H):
            wap = psaux.tile([64, 2], FP, tag="aux", name=f"wap{h}")
            nc.tensor.matmul(wap, WhT[:, h, :], a_sb[:, h, :], start=True,
                             stop=True)
            nc.scalar.copy(wa_all[:, h, :], wap)

        def shared_prep(ltag, s12, W_bf_parts, wa1_src):
            """exps + s1b + hext + uv for one attention layer.
            W_bf_parts: (xTbf_part, W_part) contraction pairs for hext;
            wa1_src: (xTbf_part, wa1-row) contraction pairs for s1b."""
            es2 = prep.tile([P, NCH], FP, tag="es2", name=f"es2_{ltag}")
            nc.scalar.activation(es2, s12[:, :, 1], AF.Exp)
            es02 = prep.tile([P, NCH], FP, tag="es02", name=f"es02_{ltag}")
            nc.scalar.activation(es02, s12[:, :, 1], AF.Exp, scale=ALPHA)
            g = prep.tile([P, NCH], FP, tag="g", name=f"g_{ltag}")
            nc.scalar.activation(g, s12[:, :, 0], AF.Exp, scale=1.0 - ALPHA)

            # s1b (bf16, mask input only): s1 row replicated via ones x wa1
            s1b = prep.tile([P, N], BF, tag="s1b", name=f"s1b_{ltag}")
            for r in range(4):
                ps = psaux.tile([P, 512], FP, tag="aux")
                for ki, (xbf, w1b) in enumerate(wa1_src):
                    nc.tensor.matmul(ps, w1b, xbf[:, r * 512:(r + 1) * 512],
                                     start=(ki == 0),
                                     stop=(ki == len(wa1_src) - 1))
                nc.scalar.copy(s1b[:, r * 512:(r + 1) * 512], ps)

            # h natural (+ones col), bf16, evacuated in 4-chunk batches
            hext = prep.tile([P, NCH, DEXT], BF, tag="hext", name=f"he_{ltag}")
            nc.vector.memset(hext[:, :, D], 1.0)
            for cg in range(4):
                hp = psaux.tile([P, 4, D], FP, tag="aux", name=f"hp{ltag}{cg}")
                for k in range(4):
                    c = cg * 4 + k
                    for ki, (xbf, wbf) in enumerate(W_bf_parts):
                        nc.tensor.matmul(hp[:, k, :],
                                         xbf[:, c * P:(c + 1) * P], wbf,
                                         start=(ki == 0),
                                         stop=(ki == len(W_bf_parts) - 1))
                nc.scalar.copy(hext[:, cg * 4:(cg + 1) * 4, 0:D], hp)

            # uv = [e^{s2} hext | e^{a s2} hext] (all bf16, DVE 4x rate)
            uv = prep.tile([P, NCH, 2 * DEXT], BF, tag="uv", name=f"uv_{ltag}")
            for c in range(NCH):
                nc.vector.tensor_scalar(uv[:, c, 0:DEXT], hext[:, c, :],
                                        es2[:, c:c + 1], None, OP.mult)
                nc.vector.tensor_scalar(uv[:, c, DEXT:], hext[:, c, :],
                                        es02[:, c:c + 1], None, OP.mult)
            if DEBUG and ltag == "h0":
                nc.sync.dma_start(out=dbg_s12[:, :, :], in_=s12)
                nc.gpsimd.dma_start(out=dbg_s1b[:, :], in_=s1b)
                nc.gpsimd.dma_start(out=dbg_uv[:, :, :], in_=uv)
            return s1b, hext, g, uv

        # ---- layer 1: four heads -> xc01/xc23 (split so the layer-2
        # transposes of head-pair 0/1 need not wait for heads 2/3) ----
        xc01 = const.tile([P, NCH, 2, D], FP)
        xc23 = const.tile([P, NCH, 2, D], FP)

        for h in range(H):
            wa = wa_all[:, h, :]
            # s12 columns (batched copies, 4 chunks per PSUM tile)
            s12 = prep.tile([P, NCH, 2], FP, tag="s12", name=f"s12_{h}")
            for cg in range(4):
                sp = psaux.tile([P, 8], FP, tag="aux", name=f"sp{h}_{cg}")
                for k in range(4):
                    c = cg * 4 + k
                    nc.tensor.matmul(sp[:, 2 * k:2 * k + 2],
                                     xT[:, c * P:(c + 1) * P], wa,
                                     start=True, stop=True)
                nc.scalar.copy(s12[:, cg * 4:(cg + 1) * 4, :], sp)

            wa1b = prep.tile([64, P], BF, tag="wa1b", name=f"wa1b_{h}")
            nc.vector.tensor_scalar(wa1b, ones128[0:64, :], wa[:, 0:1], None,
                                    OP.mult)
            s1b, hext, g, uv = shared_prep(
                f"h{h}", s12, [(xT_bf, Wh_bf[:, h, :])], [(xT_bf, wa1b)])

            def l1_out(nsum, rz, q, h=h):
                xc = xc01 if h < 2 else xc23
                dst = xc[:, q * 4:(q + 1) * 4, h % 2, :]
                _elu_norm_q(nc, wide, True, nsum, rz, q, dst, f"h{h}")

            _attention(nc, pools, scratch, s12, s1b, hext, g, uv, f"h{h}",
                       l1_out)

        if DEBUG:
            nc.sync.dma_start(out=dbg_xc[:, :, 0, :, :], in_=xc01)
            nc.sync.dma_start(out=dbg_xc[:, :, 1, :, :], in_=xc23)

        # ---- transpose xc -> xcT_bf [P, 2, N] (feature-major, bf16 only:
        # the 2e-2 tolerance admits bf16 layer-2 scores, and dropping the
        # fp32 copy frees 16KB/partition for mask-row prefetch) ----
        xcT_bf = const.tile([P, 2, N], BF)
        for c in range(NCH):
            for kc, xc in ((0, xc01), (1, xc23)):
                tp = psaux.tile([P, P], FP, tag="aux")
                nc.tensor.transpose(tp, xc[:, c, :, :], ident)
                # alternate evac engines: ACT is busy with the last heads'
                # epilogue work in this region
                if (c + kc) % 2 == 0:
                    nc.vector.tensor_copy(xcT_bf[:, kc, c * P:(c + 1) * P], tp)
                else:
                    nc.scalar.copy(xcT_bf[:, kc, c * P:(c + 1) * P], tp)

        # ---- layer 2 projections ----
        wa2 = prep.tile([P, 2, 2], FP, tag="wa2")
        wa2_bf = prep.tile([P, 2, 2], BF, tag="wa2_bf")
        for kc in range(2):
            wap = psaux.tile([P, 2], FP, tag="aux", name=f"wap2_{kc}")
            nc.tensor.matmul(wap, WoT[:, kc, :], ao, start=True, stop=True)
            nc.scalar.copy(wa2[:, kc, :], wap)
            nc.vector.tensor_copy(wa2_bf[:, kc, :], wa2[:, kc, :])

        s12_2 = prep.tile([P, NCH, 2], FP, tag="s12", name="s12_l2")
        for cg in range(4):
            sp = psaux.tile([P, 8], FP, tag="aux", name=f"sp2_{cg}")
            for k in range(4):
                c = cg * 4 + k
                for kc in range(2):
                    nc.tensor.matmul(sp[:, 2 * k:2 * k + 2],
                                     xcT_bf[:, kc, c * P:(c + 1) * P],
                                     wa2_bf[:, kc, :],
                                     start=(kc == 0), stop=(kc == 1))
            nc.scalar.copy(s12_2[:, cg * 4:(cg + 1) * 4, :], sp)

        wa1b2 = prep.tile([P, 2, P], BF, tag="wa1b2")
        for kc in range(2):
            nc.vector.tensor_scalar(wa1b2[:, kc, :], ones128, wa2[:, kc, 0:1],
                                    None, OP.mult)
        s1b_2, h2ext, g_2, uv_2 = shared_prep(
            "l2", s12_2,
            [(xcT_bf[:, 0, :], Wo_bf[:, 0, :]), (xcT_bf[:, 1, :], Wo_bf[:, 1, :])],
            [(xcT_bf[:, 0, :], wa1b2[:, 0, :]), (xcT_bf[:, 1, :], wa1b2[:, 1, :])])

        # ---- layer 2 attention + elu + log_softmax -> out (chunked DMA) ----
        out_r = out.rearrange("(c p) d -> p c d", p=P)
        o2_all = x_sb  # x_sb is dead after the startup transposes; reuse
        esum_all = const.tile([P, NCH], FP)

        def l2_out(nsum, rz, q):
            # per quarter: elu + raw exp-sum (elu output is <= ~20, so exp is
            # fp32-safe without max subtraction); Ln + final subtract deferred
            # so the Exp/Ln ACT tables swap once, not per quarter
            o2 = o2_all[:, q * 4:(q + 1) * 4, :]
            _elu_norm_q(nc, wide, False, nsum, rz, q, o2, "l2")
            escr = wide.tile([P, 4, D], FP, tag="escr", name=f"escr{q}")
            for k in range(4):
                ic = q * 4 + k
                nc.scalar.activation(escr[:, k, :], o2[:, k, :], AF.Exp,
                                     accum_out=esum_all[:, ic:ic + 1])

        _attention(nc, pools, scratch, s12_2, s1b_2, h2ext, g_2, uv_2, "l2",
                   l2_out)

        if DEBUG:
            nc.sync.dma_start(out=dbg_o2[:, :, :], in_=o2_all)

        lse = wide.tile([P, NCH], FP, tag="lse")
        nc.scalar.activation(lse, esum_all, AF.Ln)
        out_w = const.tile([P, NCH, D], FP)
        for q in range(4):
            qs = slice(q * 4, (q + 1) * 4)
            for k in range(4):
                ic = q * 4 + k
                eng = nc.vector if k % 2 == 0 else nc.gpsimd
                eng.tensor_scalar(out_w[:, ic, :], o2_all[:, ic, :],
                                  lse[:, ic:ic + 1], None, OP.subtract)
            nc.sync.dma_start(out=out_r[:, qs, :], in_=out_w[:, qs, :])

    nc.compile()
    return nc


_NC_CACHE = {}


def _make_runner(nc):
    """Build a cached sharded executable (run_bass_kernel_spmd re-traces
    jax.jit on every call; this jits once and reuses)."""
    import jax
    from jax.sharding import Mesh, PartitionSpec
    try:
        from jax.experimental.shard_map import shard_map
    except ImportError:
        from jax.shard_map import shard_map
    import concourse.mybir as mb
    from concourse import bass2jax

    bass2jax.install_neuronx_cc_hook()

    part_name = nc.partition_id_tensor.name if nc.partition_id_tensor else None
    in_names, out_names, out_avals = [], [], []
    for alloc in nc.m.functions[0].allocations:
        if not isinstance(alloc, mb.MemoryLocationSet):
            continue
        name = alloc.memorylocations[0].name
        if alloc.kind == "ExternalInput":
            if name != part_name:
                in_names.append(name)
        elif alloc.kind == "ExternalOutput":
            out_names.append(name)
            out_avals.append(jax.core.ShapedArray(
                tuple(alloc.tensor_shape), mb.dt.np(alloc.dtype)))
    n_params = len(in_names)
    all_names = in_names + out_names
    if part_name is not None:
        all_names = all_names + [part_name]

    def _body(*args):
        operands = list(args)
        if part_name is not None:
            operands.append(bass2jax.partition_id_tensor())
        return tuple(bass2jax._bass_exec_p.bind(
            *operands, out_avals=tuple(out_avals), in_names=tuple(all_names),
            out_names=tuple(out_names), lowering_input_output_aliases=(),
            sim_require_finite=True, sim_require_nnan=True, nc=nc))

    devices = jax.devices()[:B]
    mesh = Mesh(np.asarray(devices), ("core",))
    n_outs = len(out_names)
    sharded = jax.jit(
        shard_map(_body, mesh=mesh,
                  in_specs=(PartitionSpec("core"),) * (n_params + n_outs),
                  out_specs=(PartitionSpec("core"),) * n_outs,
                  check_rep=False),
        donate_argnums=tuple(range(n_params, n_params + n_outs)),
        keep_unused=True)

    def run(in_maps):
        concat_in = [
            np.concatenate([np.asarray(in_maps[c][nm])[None] for c in range(B)],
                           axis=0).reshape(B * in_maps[0][nm].shape[0],
                                           *in_maps[0][nm].shape[1:])
            for nm in in_names
        ]
        concat_zeros = [
            np.zeros((B * av.shape[0], *av.shape[1:]), av.dtype)
            for av in out_avals
        ]
        out_arrs = sharded(*concat_in, *concat_zeros)
        return [
            {nm: np.asarray(out_arrs[i]).reshape(B, *out_avals[i].shape)[c]
             for i, nm in enumerate(out_names)}
            for c in range(B)
        ]

    return run


def kernel(**inputs):
    h_states = np.ascontiguousarray(np.asarray(inputs["h_states"], dtype=np.float32))
    W_heads = np.ascontiguousarray(np.asarray(inputs["W_heads"], dtype=np.float32))
    a_heads = np.ascontiguousarray(np.asarray(inputs["a_heads"], dtype=np.float32))
    W_out = np.ascontiguousarray(np.asarray(inputs["W_out"], dtype=np.float32))
    a_out = np.ascontiguousarray(np.asarray(inputs["a_out"], dtype=np.float32))

    if "nc" not in _NC_CACHE:
        _NC_CACHE["nc"] = build_kernel()
        _NC_CACHE["run"] = _make_runner(_NC_CACHE["nc"])

    xs = h_states.reshape(B, N, D)
    in_maps = [
        {"x": xs[c], "W_heads": W_heads, "a_heads": a_heads,
         "W_out": W_out, "a_out": a_out}
        for c in range(B)
    ]
    results = _NC_CACHE["run"](in_maps)
    return np.concatenate([results[c]["out"] for c in range(B)], axis=0)


if __name__ == "__main__":
    # smoke test (self-contained: random inputs, shape/dtype check only)
    rng = np.random.default_rng(0)
    inputs = {
        "h_states": rng.standard_normal((B * N, D)).astype(np.float32),
        "W_heads": rng.standard_normal((H, D, D)).astype(np.float32) * 0.18,
        "a_heads": rng.standard_normal((H, 2 * D)).astype(np.float32) * 0.18,
        "W_out": rng.standard_normal((H * D, D)).astype(np.float32) * 0.09,
        "a_out": rng.standard_normal((2 * D,)).astype(np.float32) * 0.18,
        "seq_start_end": (np.arange(B, dtype=np.int32)[:, None] * N
                          + np.array([0, N], dtype=np.int32)[None, :]),
    }
    got = kernel(**inputs)
    print("kernel output", got.shape, got.dtype)


# revision 46
# speedup vs baseline: 1.1026x; 1.0285x over previous
"""Bass/Tile Trainium2 kernel for a 2-layer dense multi-head GAT over a batch
of B=8 independent subgraphs (2048 nodes each, equal contiguous segments).

Sharding: one subgraph per NeuronCore (8 cores), parameters replicated.

Algorithm (per core / subgraph, per attention layer):
  scores are rank-1:  e_ij = leaky_relu(s1_i + s2_j),  s1 = h@a1, s2 = h@a2.
  exp(leaky_relu(t)) is separable through the sign mask M_ij = [s1_i+s2_j>=0]:
      p_ij = M_ij e^{s1_i} e^{s2_j} + (1-M_ij) e^{a s1_i} e^{a s2_j}
  so softmax(e) @ h needs NO N^2 exp work:
      num_i = g_i (M @ u)_i - ((M @ v) - vtot)_i           (e^{a s1} cancels in
      u_j = e^{s2_j} [h_j|1],  v_j = e^{a s2_j} [h_j|1],    the Z ratio; g =
      out_i = num_i[:64] / num_i[64]                        e^{(1-a) s1})
  The N^2 work is one DVE compare pass per (layer, j-chunk) producing a full
  [128, N] 0/1 bf16 mask row reused by all four i-quarters, plus bf16 mask
  matmuls (single stream, no residual: the 2e-2 tolerance leaves plenty of
  room).  vtot seeds the PSUM accumulator (bf16 hi+res rows, K=2 matmul), so
  A[:, DEXT:] = M@v - vtot directly and the epilogue is one fused op chain:
  ACT evacuates A, Pool computes nsum = g*Au - Av', DVE does the normalize+elu
  min/max in two-op tensor_scalars at bf16 4x rate.
"""

from contextlib import ExitStack

import numpy as np

import concourse.bass as bass
import concourse.tile as tile
from concourse import bacc, mybir
from concourse.masks import make_identity

FP = mybir.dt.float32
BF = mybir.dt.bfloat16
AF = mybir.ActivationFunctionType
OP = mybir.AluOpType

B = 8
N = 2048
D = 64
H = 4
ALPHA = 0.2
P = 128
NCH = N // P  # 16 chunks of 128 nodes
DEXT = D + 1  # h plus ones column

# full mask rows generated on Pool (GpSimd) instead of DVE, per layer
POOL_MASK_JCS = (14, 15)
DEBUG = False


def _attention(nc, pools, scratch, s12, s1b, hext, g, uv, ltag, out_cb):
    """Dense-GAT attention layer: out = softmax(lrelu(s1_i+s2_j)) @ h.

    s12:  [P, NCH, 2] SBUF f32 (s1|s2 in node-chunk column layout)
    s1b:  [P, N] SBUF bf16 (s1 replicated across partitions, free dim = node)
    hext: [P, NCH, DEXT] SBUF bf16 (h natural, col D == 1.0)
    g:    [P, NCH] SBUF f32 (e^{(1-a) s1})
    uv:   [P, NCH, 2*DEXT] SBUF bf16 ([e^{s2} hext | e^{a s2} hext])
    out_cb(nsum, rz, q): consumes quarter q ([P,4,DEXT] f32 + [P,4] recip).
    """
    const, prep, mask_pool, wide, small, psA, psaux = pools
    ones_col_bf = scratch["ones_col_bf"]
    ones_row_bf = scratch["ones_row_bf"]

    # --- vtot row: [1, 130] = [0...0 | -sum_j v_j] as bf16 hi+res, stacked
    # [2, 130] via a DMA hop so each accumulator is seeded by one K=2 matmul
    # (PSUM accumulation is order-insensitive; seeding happens last) ---
    vt_ps = psaux.tile([1, DEXT], FP, tag="aux", name=f"vt{ltag}")
    for c in range(NCH):
        nc.tensor.matmul(vt_ps, ones_col_bf, uv[:, c, DEXT:],
                         start=(c == 0), stop=(c == NCH - 1))
    vrow_bf = prep.tile([1, 2 * DEXT], BF, tag="vrow_bf")
    nc.vector.memset(vrow_bf[:, 0:DEXT], 0.0)
    nc.vector.tensor_scalar(vrow_bf[:, DEXT:], vt_ps, -1.0, None, OP.mult)
    vres = prep.tile([1, DEXT], BF, tag="vres")
    nc.vector.scalar_tensor_tensor(vres, vt_ps, -1.0, vrow_bf[:, DEXT:],
                                   OP.mult, OP.subtract)
    vrow2 = prep.tile([2, 2 * DEXT], BF, tag="vrow2")
    nc.sync.dma_start(out=vrow2[0:1, :], in_=vrow_bf)
    nc.sync.dma_start(out=vrow2[1:2, 0:DEXT], in_=vrow_bf[:, 0:DEXT])
    nc.sync.dma_start(out=vrow2[1:2, DEXT:], in_=vres)

    # --- full-row masks [128 j, N i], one per j-chunk, reused by all four
    # i-quarters.  A couple of rows go to the otherwise-idle Pool engine. ---
    mask_rows = []
    for jc in range(NCH):
        mt = mask_pool.tile([P, N], BF, tag="mrow", name=f"m{ltag}_{jc}")
        eng = nc.gpsimd if jc in POOL_MASK_JCS else nc.vector
        eng.tensor_scalar(mt, s1b, s12[:, jc, 1:2], 0.0, OP.add, OP.is_ge)
        mask_rows.append(mt)
    if scratch.get("dbg_m0") is not None and ltag == "h0":
        nc.gpsimd.dma_start(out=scratch["dbg_m0"][:, :], in_=mask_rows[0])
        nc.sync.dma_start(out=scratch["dbg_g"][:, :], in_=g)

    # --- masked attention matmuls + per-quarter epilogue ---
    for q in range(4):  # quarters of the i (destination-node) axis
        # one PSUM bank per il: interleaved accumulation chains must not
        # share a bank
        A = [psA.tile([P, 2 * DEXT], FP, tag="A", name=f"A{ltag}_{q}_{il}")
             for il in range(4)]
        for jc in range(NCH):
            mt = mask_rows[jc]
            for il in range(4):
                sl = mt[:, q * 512 + il * P: q * 512 + (il + 1) * P]
                nc.tensor.matmul(A[il], sl, uv[:, jc, :],
                                 start=(jc == 0), stop=False)
        for il in range(4):
            nc.tensor.matmul(A[il], ones_row_bf[0:2, :], vrow2,
                             start=False, stop=True)
        # epilogue: ACT evacuates A (keeps the DVE STT off PSUM, whose access
        # latency costs more than the extra ACT copy), then DVE folds g:
        # nsum = g*Au - Av', freeing the A banks
        nsum = wide.tile([P, 4, DEXT], FP, tag="nsum", name=f"ns{ltag}_{q}")
        for il in range(4):
            ic = q * 4 + il
            Asb = small.tile([P, 2 * DEXT], FP, tag="Asb")
            nc.scalar.copy(Asb, A[il])
            nc.vector.scalar_tensor_tensor(
                nsum[:, il, :], Asb[:, 0:DEXT],
                g[:, ic:ic + 1], Asb[:, DEXT:], OP.mult, OP.subtract)
        if DEBUG and ltag == "h0":
            dbg = scratch["dbg_ns"]
            nc.sync.dma_start(out=dbg[q], in_=nsum)
        rz = small.tile([P, 4], FP, tag="rz")
        nc.vector.reciprocal(rz, nsum[:, :, D])
        out_cb(nsum, rz, q)


def _elu_norm_q(nc, wide, pool_comb, nsum, rz, q, dst, ltag):
    """dst[:, q*4:(q+1)*4, ...] = elu(nsum[:, k, 0:D] * rz[:, k]) fused:
    r/m two-op tensor_scalars (bf16 out, DVE), exp on ACT, combine on Pool."""
    r = wide.tile([P, 4, D], BF, tag="elu_r", name=f"er{ltag}{q}")
    m = wide.tile([P, 4, D], BF, tag="elu_m", name=f"em{ltag}{q}")
    for k in range(4):
        nc.vector.tensor_scalar(r[:, k, :], nsum[:, k, 0:D], rz[:, k:k + 1],
                                0.0, OP.mult, OP.max)
        nc.vector.tensor_scalar(m[:, k, :], nsum[:, k, 0:D], rz[:, k:k + 1],
                                0.0, OP.mult, OP.min)
    e = wide.tile([P, 4, D], BF, tag="elu_e", name=f"ee{ltag}{q}")
    nc.scalar.activation(e, m, AF.Exp)
    # dst = (e + (-1)) + r   (scalar_tensor_tensor is not a legal Pool opcode)
    nc.vector.scalar_tensor_tensor(dst, e, -1.0, r, OP.add, OP.add)


def build_kernel():
    nc = bacc.Bacc("TRN2", target_bir_lowering=False, debug=False,
                   num_devices=B)

    x = nc.dram_tensor("x", [N, D], FP, kind="ExternalInput")
    W_heads = nc.dram_tensor("W_heads", [H, D, D], FP, kind="ExternalInput")
    a_heads = nc.dram_tensor("a_heads", [H, 2 * D], FP, kind="ExternalInput")
    W_out = nc.dram_tensor("W_out", [H * D, D], FP, kind="ExternalInput")
    a_out = nc.dram_tensor("a_out", [2 * D], FP, kind="ExternalInput")
    out = nc.dram_tensor("out", [N, D], FP, kind="ExternalOutput")
    dbg_xc = nc.dram_tensor("dbg_xc", [P, NCH, 2, 2, D], FP,
                            kind="ExternalOutput") if DEBUG else None
    dbg_o2 = nc.dram_tensor("dbg_o2", [P, NCH, D], FP,
                            kind="ExternalOutput") if DEBUG else None
    dbg_s12 = nc.dram_tensor("dbg_s12", [P, NCH, 2], FP,
                             kind="ExternalOutput") if DEBUG else None
    dbg_uv = nc.dram_tensor("dbg_uv", [P, NCH, 2 * DEXT], FP,
                            kind="ExternalOutput") if DEBUG else None
    dbg_s1b = nc.dram_tensor("dbg_s1b", [P, N], FP,
                             kind="ExternalOutput") if DEBUG else None
    dbg_ns = nc.dram_tensor("dbg_ns", [4, P, 4, DEXT], FP,
                            kind="ExternalOutput") if DEBUG else None
    dbg_A = nc.dram_tensor("dbg_A", [4, P, 4, 2 * DEXT], FP,
                           kind="ExternalOutput") if DEBUG else None
    dbg_m0 = nc.dram_tensor("dbg_m0", [P, N], FP,
                            kind="ExternalOutput") if DEBUG else None
    dbg_g = nc.dram_tensor("dbg_g", [P, NCH], FP,
                           kind="ExternalOutput") if DEBUG else None

    with tile.TileContext(nc) as tc, ExitStack() as ctx:
        const = ctx.enter_context(tc.tile_pool(name="const", bufs=1))
        prep = ctx.enter_context(tc.tile_pool(name="prep", bufs=3))
        mask_pool = ctx.enter_context(tc.tile_pool(name="mask", bufs=22))
        wide = ctx.enter_context(tc.tile_pool(name="wide", bufs=3))
        small = ctx.enter_context(tc.tile_pool(name="small", bufs=6))
        psA = ctx.enter_context(tc.tile_pool(name="psA", bufs=4, space="PSUM"))
        psaux = ctx.enter_context(tc.tile_pool(name="psaux", bufs=4, space="PSUM"))
        pools = (const, prep, mask_pool, wide, small, psA, psaux)

        ident = const.tile([P, P], FP)
        make_identity(nc, ident)
        ones128 = const.tile([P, P], FP)
        nc.vector.memset(ones128, 1.0)
        ones_col_bf = const.tile([P, 1], BF)
        nc.vector.memset(ones_col_bf, 1.0)
        ones_row_bf = const.tile([2, P], BF)
        nc.vector.memset(ones_row_bf, 1.0)
        scratch = {"ones128": ones128, "ones_col_bf": ones_col_bf,
                   "ones_row_bf": ones_row_bf, "dbg_ns": dbg_ns,
                   "dbg_A": dbg_A, "dbg_m0": dbg_m0, "dbg_g": dbg_g}

        # ---- load inputs (x in 4 pieces so transposes start early) ----
        x_sb = const.tile([P, NCH, D], FP)
        x_r = x.rearrange("(c p) d -> p c d", p=P)
        for r4 in range(4):
            nc.sync.dma_start(out=x_sb[:, r4 * 4:(r4 + 1) * 4, :],
                              in_=x_r[:, r4 * 4:(r4 + 1) * 4, :])
        Wh = const.tile([64, H, D], FP)
        nc.sync.dma_start(out=Wh, in_=W_heads.rearrange("h k d -> k h d"))
        WhT = const.tile([64, H, D], FP)
        nc.sync.dma_start(out=WhT, in_=W_heads.rearrange("h k d -> d h k"))
        a_sb = const.tile([64, H, 2], FP)
        nc.sync.dma_start(out=a_sb, in_=a_heads.rearrange("h (t k) -> k h t", t=2))
        Wo = const.tile([P, 2, D], FP)
        nc.sync.dma_start(out=Wo, in_=W_out.rearrange("(c k) d -> k c d", k=P))
        WoT = const.tile([64, 2, P], FP)
        nc.sync.dma_start(out=WoT, in_=W_out.rearrange("(c k) d -> d c k", k=P))
        ao = const.tile([64, 2], FP)
        nc.sync.dma_start(out=ao, in_=a_out.rearrange("(t k) -> k t", t=2))

        # bf16 weight shadows for the payload-path matmuls
        Wh_bf = const.tile([64, H, D], BF)
        nc.gpsimd.tensor_copy(Wh_bf, Wh)
        Wo_bf = const.tile([P, 2, D], BF)
        nc.gpsimd.tensor_copy(Wo_bf, Wo)

        # ---- xT via PE transposes; bf16 shadow (Pool, in pieces) ----
        xT = const.tile([64, N], FP)
        for c in range(NCH):
            tp = psaux.tile([64, P], FP, tag="aux")
            nc.tensor.transpose(tp, x_sb[:, c, :], ident)
            # alternate evac engines so ACT is free for the head-0 prep chain
            if c % 2 == 0:
                nc.vector.tensor_copy(xT[:, c * P:(c + 1) * P], tp)
            else:
                nc.scalar.copy(xT[:, c * P:(c + 1) * P], tp)
        xT_bf = const.tile([64, N], BF)
        for r in range(4):
            nc.gpsimd.tensor_copy(xT_bf[:, r * 512:(r + 1) * 512],
                                  xT[:, r * 512:(r + 1) * 512])

        # all heads' wa = W_h @ [a1|a2] upfront (re-association: s = x @ wa);
        # only needs the parameter DMAs, so it fills the startup bubble
        wa_all = const.tile([64, H, 2], FP)
        for h in range(H):
            wap = psaux.tile([64, 2], FP, tag="aux", name=f"wap{h}")
            nc.tensor.matmul(wap, WhT[:, h, :], a_sb[:, h, :], start=True,
                             stop=True)
            nc.scalar.copy(wa_all[:, h, :], wap)

        def shared_prep(ltag, s12, W_bf_parts, wa1_src):
            """exps + s1b + hext + uv for one attention layer.
            W_bf_parts: (xTbf_part, W_part) contraction pairs for hext;
            wa1_src: (xTbf_part, wa1-row) contraction pairs for s1b."""
            es2 = prep.tile([P, NCH], FP, tag="es2", name=f"es2_{ltag}")
            nc.scalar.activation(es2, s12[:, :, 1], AF.Exp)
            es02 = prep.tile([P, NCH], FP, tag="es02", name=f"es02_{ltag}")
            nc.scalar.activation(es02, s12[:, :, 1], AF.Exp, scale=ALPHA)
            g = prep.tile([P, NCH], FP, tag="g", name=f"g_{ltag}")
            nc.scalar.activation(g, s12[:, :, 0], AF.Exp, scale=1.0 - ALPHA)

            # s1b (bf16, mask input only): s1 row replicated via ones x wa1
            s1b = prep.tile([P, N], BF, tag="s1b", name=f"s1b_{ltag}")
            for r in range(4):
                ps = psaux.tile([P, 512], FP, tag="aux")
                for ki, (xbf, w1b) in enumerate(wa1_src):
                    nc.tensor.matmul(ps, w1b, xbf[:, r * 512:(r + 1) * 512],
                                     start=(ki == 0),
                                     stop=(ki == len(wa1_src) - 1))
                nc.scalar.copy(s1b[:, r * 512:(r + 1) * 512], ps)

            # h natural (+ones col), bf16, evacuated in 4-chunk batches
            hext = prep.tile([P, NCH, DEXT], BF, tag="hext", name=f"he_{ltag}")
            nc.vector.memset(hext[:, :, D], 1.0)
            for cg in range(4):
                hp = psaux.tile([P, 4, D], FP, tag="aux", name=f"hp{ltag}{cg}")
                for k in range(4):
                    c = cg * 4 + k
                    for ki, (xbf, wbf) in enumerate(W_bf_parts):
                        nc.tensor.matmul(hp[:, k, :],
                                         xbf[:, c * P:(c + 1) * P], wbf,
                                         start=(ki == 0),
                                         stop=(ki == len(W_bf_parts) - 1))
                nc.scalar.copy(hext[:, cg * 4:(cg + 1) * 4, 0:D], hp)

            # uv = [e^{s2} hext | e^{a s2} hext] (all bf16, DVE 4x rate)
            uv = prep.tile([P, NCH, 2 * DEXT], BF, tag="uv", name=f"uv_{ltag}")
            for c in range(NCH):
                nc.vector.tensor_scalar(uv[:, c, 0:DEXT], hext[:, c, :],
                                        es2[:, c:c + 1], None, OP.mult)
                nc.vector.tensor_scalar(uv[:, c, DEXT:], hext[:, c, :],
                                        es02[:, c:c + 1], None, OP.mult)
            if DEBUG and ltag == "h0":
                nc.sync.dma_start(out=dbg_s12[:, :, :], in_=s12)
                nc.gpsimd.dma_start(out=dbg_s1b[:, :], in_=s1b)
                nc.gpsimd.dma_start(out=dbg_uv[:, :, :], in_=uv)
            return s1b, hext, g, uv

        # ---- layer 1: four heads -> xc01/xc23 (split so the layer-2
        # transposes of head-pair 0/1 need not wait for heads 2/3) ----
        xc01 = const.tile([P, NCH, 2, D], FP)
        xc23 = const.tile([P, NCH, 2, D], FP)

        for h in range(H):
            wa = wa_all[:, h, :]
            # s12 columns (batched copies, 4 chunks per PSUM tile)
            s12 = prep.tile([P, NCH, 2], FP, tag="s12", name=f"s12_{h}")
            for cg in range(4):
                sp = psaux.tile([P, 8], FP, tag="aux", name=f"sp{h}_{cg}")
                for k in range(4):
                    c = cg * 4 + k
                    nc.tensor.matmul(sp[:, 2 * k:2 * k + 2],
                                     xT[:, c * P:(c + 1) * P], wa,
                                     start=True, stop=True)
                nc.scalar.copy(s12[:, cg * 4:(cg + 1) * 4, :], sp)

            wa1b = prep.tile([64, P], BF, tag="wa1b", name=f"wa1b_{h}")
            nc.vector.tensor_scalar(wa1b, ones128[0:64, :], wa[:, 0:1], None,
                                    OP.mult)
            s1b, hext, g, uv = shared_prep(
                f"h{h}", s12, [(xT_bf, Wh_bf[:, h, :])], [(xT_bf, wa1b)])

            def l1_out(nsum, rz, q, h=h):
                xc = xc01 if h < 2 else xc23
                dst = xc[:, q * 4:(q + 1) * 4, h % 2, :]
                _elu_norm_q(nc, wide, True, nsum, rz, q, dst, f"h{h}")

            _attention(nc, pools, scratch, s12, s1b, hext, g, uv, f"h{h}",
                       l1_out)

        if DEBUG:
            nc.sync.dma_start(out=dbg_xc[:, :, 0, :, :], in_=xc01)
            nc.sync.dma_start(out=dbg_xc[:, :, 1, :, :], in_=xc23)

        # ---- transpose xc -> xcT_bf [P, 2, N] (feature-major, bf16 only:
        # the 2e-2 tolerance admits bf16 layer-2 scores, and dropping the
        # fp32 copy frees 16KB/partition for mask-row prefetch) ----
        xcT_bf = const.tile([P, 2, N], BF)
        for c in range(NCH):
            for kc, xc in ((0, xc01), (1, xc23)):
                tp = psaux.tile([P, P], FP, tag="aux")
                nc.tensor.transpose(tp, xc[:, c, :, :], ident)
                # alternate evac engines: ACT is busy with the last heads'
                # epilogue work in this region
                if (c + kc) % 2 == 0:
                    nc.vector.tensor_copy(xcT_bf[:, kc, c * P:(c + 1) * P], tp)
                else:
                    nc.scalar.copy(xcT_bf[:, kc, c * P:(c + 1) * P], tp)

        # ---- layer 2 projections ----
        wa2 = prep.tile([P, 2, 2], FP, tag="wa2")
        wa2_bf = prep.tile([P, 2, 2], BF, tag="wa2_bf")
        for kc in range(2):
            wap = psaux.tile([P, 2], FP, tag="aux", name=f"wap2_{kc}")
            nc.tensor.matmul(wap, WoT[:, kc, :], ao, start=True, stop=True)
            nc.scalar.copy(wa2[:, kc, :], wap)
            nc.vector.tensor_copy(wa2_bf[:, kc, :], wa2[:, kc, :])

        s12_2 = prep.tile([P, NCH, 2], FP, tag="s12", name="s12_l2")
        for cg in range(4):
            sp = psaux.tile([P, 8], FP, tag="aux", name=f"sp2_{cg}")
            for k in range(4):
                c = cg * 4 + k
                for kc in range(2):
                    nc.tensor.matmul(sp[:, 2 * k:2 * k + 2],
                                     xcT_bf[:, kc, c * P:(c + 1) * P],
                                     wa2_bf[:, kc, :],
                                     start=(kc == 0), stop=(kc == 1))
            nc.scalar.copy(s12_2[:, cg * 4:(cg + 1) * 4, :], sp)

        wa1b2 = prep.tile([P, 2, P], BF, tag="wa1b2")
        for kc in range(2):
            nc.vector.tensor_scalar(wa1b2[:, kc, :], ones128, wa2[:, kc, 0:1],
                                    None, OP.mult)
        s1b_2, h2ext, g_2, uv_2 = shared_prep(
            "l2", s12_2,
            [(xcT_bf[:, 0, :], Wo_bf[:, 0, :]), (xcT_bf[:, 1, :], Wo_bf[:, 1, :])],
            [(xcT_bf[:, 0, :], wa1b2[:, 0, :]), (xcT_bf[:, 1, :], wa1b2[:, 1, :])])

        # ---- layer 2 attention + elu + log_softmax -> out (chunked DMA) ----
        out_r = out.rearrange("(c p) d -> p c d", p=P)
        o2_all = x_sb  # x_sb is dead after the startup transposes; reuse
        esum_all = const.tile([P, NCH], FP)

        def l2_out(nsum, rz, q):
            # per quarter: elu + raw exp-sum (elu output is <= ~20, so exp is
            # fp32-safe without max subtraction); Ln + final subtract deferred
            # so the Exp/Ln ACT tables swap once, not per quarter
            o2 = o2_all[:, q * 4:(q + 1) * 4, :]
            _elu_norm_q(nc, wide, False, nsum, rz, q, o2, "l2")
            escr = wide.tile([P, 4, D], FP, tag="escr", name=f"escr{q}")
            for k in range(4):
                ic = q * 4 + k
                nc.scalar.activation(escr[:, k, :], o2[:, k, :], AF.Exp,
                                     accum_out=esum_all[:, ic:ic + 1])

        _attention(nc, pools, scratch, s12_2, s1b_2, h2ext, g_2, uv_2, "l2",
                   l2_out)

        if DEBUG:
            nc.sync.dma_start(out=dbg_o2[:, :, :], in_=o2_all)

        lse = wide.tile([P, NCH], FP, tag="lse")
        nc.scalar.activation(lse, esum_all, AF.Ln)
        out_w = const.tile([P, NCH, D], FP)
        for q in range(4):
            qs = slice(q * 4, (q + 1) * 4)
            for k in range(4):
                ic = q * 4 + k
                eng = nc.vector if k % 2 == 0 else nc.gpsimd
                eng.tensor_scalar(out_w[:, ic, :], o2_all[:, ic, :],
                                  lse[:, ic:ic + 1], None, OP.subtract)
            nc.sync.dma_start(out=out_r[:, qs, :], in_=out_w[:, qs, :])

    nc.compile()
    return nc


_NC_CACHE = {}


def _make_runner(nc):
    """Build a cached sharded executable (run_bass_kernel_spmd re-traces
    jax.jit on every call; this jits once and reuses)."""
    import jax
    from jax.sharding import Mesh, PartitionSpec
    try:
        from jax.experimental.shard_map import shard_map
    except ImportError:
        from jax.shard_map import shard_map
    import concourse.mybir as mb
    from concourse import bass2jax

    bass2jax.install_neuronx_cc_hook()

    part_name = nc.partition_id_tensor.name if nc.partition_id_tensor else None
    in_names, out_names, out_avals = [], [], []
    for alloc in nc.m.functions[0].allocations:
        if not isinstance(alloc, mb.MemoryLocationSet):
            continue
        name = alloc.memorylocations[0].name
        if alloc.kind == "ExternalInput":
            if name != part_name:
                in_names.append(name)
        elif alloc.kind == "ExternalOutput":
            out_names.append(name)
            out_avals.append(jax.core.ShapedArray(
                tuple(alloc.tensor_shape), mb.dt.np(alloc.dtype)))
    n_params = len(in_names)
    all_names = in_names + out_names
    if part_name is not None:
        all_names = all_names + [part_name]

    def _body(*args):
        operands = list(args)
        if part_name is not None:
            operands.append(bass2jax.partition_id_tensor())
        return tuple(bass2jax._bass_exec_p.bind(
            *operands, out_avals=tuple(out_avals), in_names=tuple(all_names),
            out_names=tuple(out_names), lowering_input_output_aliases=(),
            sim_require_finite=True, sim_require_nnan=True, nc=nc))

    devices = jax.devices()[:B]
    mesh = Mesh(np.asarray(devices), ("core",))
    n_outs = len(out_names)
    sharded = jax.jit(
        shard_map(_body, mesh=mesh,
                  in_specs=(PartitionSpec("core"),) * (n_params + n_outs),
                  out_specs=(PartitionSpec("core"),) * n_outs,
                  check_rep=False),
        donate_argnums=tuple(range(n_params, n_params + n_outs)),
        keep_unused=True)

    def run(in_maps):
        concat_in = [
            np.concatenate([np.asarray(in_maps[c][nm])[None] for c in range(B)],
                           axis=0).reshape(B * in_maps[0][nm].shape[0],
                                           *in_maps[0][nm].shape[1:])
            for nm in in_names
        ]
        concat_zeros = [
            np.zeros((B * av.shape[0], *av.shape[1:]), av.dtype)
            for av in out_avals
        ]
        out_arrs = sharded(*concat_in, *concat_zeros)
        return [
            {nm: np.asarray(out_arrs[i]).reshape(B, *out_avals[i].shape)[c]
             for i, nm in enumerate(out_names)}
            for c in range(B)
        ]

    return run


def kernel(**inputs):
    h_states = np.ascontiguousarray(np.asarray(inputs["h_states"], dtype=np.float32))
    W_heads = np.ascontiguousarray(np.asarray(inputs["W_heads"], dtype=np.float32))
    a_heads = np.ascontiguousarray(np.asarray(inputs["a_heads"], dtype=np.float32))
    W_out = np.ascontiguousarray(np.asarray(inputs["W_out"], dtype=np.float32))
    a_out = np.ascontiguousarray(np.asarray(inputs["a_out"], dtype=np.float32))

    if "nc" not in _NC_CACHE:
        _NC_CACHE["nc"] = build_kernel()
        _NC_CACHE["run"] = _make_runner(_NC_CACHE["nc"])

    xs = h_states.reshape(B, N, D)
    in_maps = [
        {"x": xs[c], "W_heads": W_heads, "a_heads": a_heads,
         "W_out": W_out, "a_out": a_out}
        for c in range(B)
    ]
    results = _NC_CACHE["run"](in_maps)
    return np.concatenate([results[c]["out"] for c in range(B)], axis=0)


if __name__ == "__main__":
    # smoke test (self-contained: random inputs, shape/dtype check only)
    rng = np.random.default_rng(0)
    inputs = {
        "h_states": rng.standard_normal((B * N, D)).astype(np.float32),
        "W_heads": rng.standard_normal((H, D, D)).astype(np.float32) * 0.18,
        "a_heads": rng.standard_normal((H, 2 * D)).astype(np.float32) * 0.18,
        "W_out": rng.standard_normal((H * D, D)).astype(np.float32) * 0.09,
        "a_out": rng.standard_normal((2 * D,)).astype(np.float32) * 0.18,
        "seq_start_end": (np.arange(B, dtype=np.int32)[:, None] * N
                          + np.array([0, N], dtype=np.int32)[None, :]),
    }
    got = kernel(**inputs)
    print("kernel output", got.shape, got.dtype)
